# revision 17
# baseline (speedup 1.0000x reference)
"""Trainium2 Bass kernel for the MCA (multi-axis pooled gating) module.

Computation (per sample b):
    hw_m = mean_{u,v} x   uv_m = mean_{h,w} x   uh_m = mean_{v,w} x   vw_m = mean_{u,h} x
    body = conv2(silu(conv1(uvhw)))   (1x1 convs on the packed (H+V, W+U) pooled map)
    gates: hw_g = f0(body_hw), uv_g = f1(body_uv), uh_g = f2(body_uh), vw_g = f3(body_vw)
    out = x * (hw_g + uv_g + uh_g + vw_g)      (each gate broadcast to the 6D shape)

Distribution: 8 cores = 4 samples x 2 h-halves. Each core owns
x[b, :, :, :, hh*32:(hh+1)*32, :], held in SBUF as bf16 (the host converts; the
2e-2 tolerance dwarfs bf16 rounding), so HBM traffic is halved in both
directions versus f32. The only cross-core data are the h-reduced pools
(uv_m, vw_m partials), exchanged as two small pair AllReduces (split by v so
the second half overlaps the first group's gating work).

On-core layout: SBUF partition p = hs*64 + c, where the core's 32 h-rows split
as h2 = hs*16 + hl. Pools that fully reduce h fold the hs partition halves with
a small DMA+add before the collectives.

Engine budget: PE does the (u,v)-pool accumulation (bf16 identity matmuls),
the vw hl-fold, one v's w-reduction, the channel-mixing convs and the B-group
gate-broadcast adds; DVE does the remaining w-reductions, the final multiplies
and a share of the gate adds; GpSimd takes the other gate adds plus collective
staging; ACT does PSUM evacuation, scaling, SiLU and gate biases.
"""

import sys
if '/opt/trn_rl_repo' not in sys.path:
    sys.path.insert(0, '/opt/trn_rl_repo')

from contextlib import ExitStack

import numpy as np
import concourse.bass as bass
import concourse.bacc as bacc
import concourse.tile as tile
from concourse import mybir

F32 = mybir.dt.float32
F32R = mybir.dt.float32r
BF16 = mybir.dt.bfloat16
AF = mybir.ActivationFunctionType
ALU = mybir.AluOpType

# ---- tunable routing -------------------------------------------------------
SW_PE_VS = ()              # v indices whose w-reduction runs on PE (rest DVE)
# per-chunk G-add route, cycled over the 25 (v,u) chunks
ROUTE_CYCLE = ("pe", "gp", "dve", "pe", "gp", "pe",
               "gp", "dve", "pe", "gp", "pe", "gp")


def _ap(t_ap, dims, extra_off=0):
    """Manual free-dim view of an AP: dims = [(step_elems, count), ...]."""
    return bass.AP(
        tensor=t_ap.tensor,
        offset=t_ap.offset + extra_off,
        ap=[list(t_ap.ap[0])] + [[s, c] for (s, c) in dims],
    )


def build_program(C=64, U=5, V=5, H2=32, W=64, n_cores=8):
    """One SPMD program; per-core inputs select the (b, h-half) shard."""
    assert C == 64 and H2 % 2 == 0
    HL = H2 // 2              # h rows per hs partition group
    P = 2 * C                 # 128 partitions = (hs, c)
    CHW = HL * W              # free size of one (u,v) chunk per partition
    NMM = min(512, CHW)       # matmul moving-operand max (PSUM bank)
    NUV = U * V
    NB = U + W                # per-v partials block: [uv_u | vw_w]
    VA = max(1, (3 * V) // 5)  # v-count in the first collective group
    H = 2 * H2
    FREE = U * V * HL * W // V  # per-v free size = U*HL*W

    nc = bacc.Bacc('TRN2', target_bir_lowering=False, debug=False,
                   enable_asserts=False, num_devices=n_cores)

    x_d = nc.dram_tensor("x", [V, P, U, HL, W], BF16, kind="ExternalInput").ap()
    out_d = nc.dram_tensor("out", [V, P, U, HL, W], BF16,
                           kind="ExternalOutput").ap()
    NCON = P + 6 * C
    cpack_d = nc.dram_tensor("cpack", [P, NCON], BF16, kind="ExternalInput").ap()
    cbias_d = nc.dram_tensor("cbias", [C, 8], F32, kind="ExternalInput").ap()

    with tile.TileContext(nc) as tc, ExitStack() as ctx:
        consts = ctx.enter_context(tc.tile_pool(name="consts", bufs=1))
        xpool = ctx.enter_context(tc.tile_pool(name="x", bufs=V))
        sumu_pool = ctx.enter_context(tc.tile_pool(name="sumu", bufs=2))
        small = ctx.enter_context(tc.tile_pool(name="small", bufs=1))
        convp = ctx.enter_context(tc.tile_pool(name="convp", bufs=2))
        ppool = ctx.enter_context(tc.tile_pool(name="pp", bufs=U))
        gpool = ctx.enter_context(tc.tile_pool(name="gpool", bufs=3))
        opool = ctx.enter_context(tc.tile_pool(name="opool", bufs=2))
        phase1_ctx = ExitStack()
        ps_acc = phase1_ctx.enter_context(
            tc.tile_pool(name="ps_acc", bufs=3, space="PSUM"))
        ps_hw = phase1_ctx.enter_context(
            tc.tile_pool(name="ps_hw", bufs=1, space="PSUM"))
        dram = ctx.enter_context(tc.tile_pool(name="dram", bufs=1, space="DRAM"))

        cpack = consts.tile([P, NCON], BF16)
        nc.gpsimd.dma_start(out=cpack[:], in_=cpack_d[:, :])
        cbias = consts.tile([C, 8], F32)
        nc.gpsimd.dma_start(out=cbias[:], in_=cbias_d[:, :])
        id16 = cpack[:, 0:P]
        # weights replicated on both hs partition halves so conv matmuls can
        # pick an lhsT whose base partition matches the rhs half
        wnames = ("w1T", "w2T", "f0T", "f1T", "f2T", "f3T")
        wt = {nm: cpack[:, P + i * C:P + (i + 1) * C]
              for i, nm in enumerate(wnames)}
        bnames = ("b1", "b2", "fb0", "fb1", "fb2", "fb3")
        bt = {nm: cbias[0:C, i:i + 1] for i, nm in enumerate(bnames)}

        def mm16(out_ps, rhs, start, stop):
            nc.tensor.matmul(out_ps, id16, rhs, start=start, stop=stop)

        def mmw(out_ps, lhsT, rhs, start=True, stop=True):
            nc.tensor.matmul(out_ps, lhsT, rhs, start=start, stop=stop)

        # ---------------- Phase 1: load x + pools -------------------------
        partials = small.tile([P, V * NB], F32)   # per-v blocks [uv_u | vw_w]
        s_w = small.tile([P, V, U, HL], F32)      # x summed over w
        hw_ps = ps_hw.tile([P, CHW], F32)         # x summed over (u, v)
        xv_t = []

        cc_out_sb = {}
        cc_out_d = {}

        def emit_group_cc(g, v0, v1):
            """Fold hs halves of partials[v0:v1]; trigger the pair AllReduce."""
            sl = slice(v0 * NB, v1 * NB)
            n = (v1 - v0) * NB
            ft = small.tile([C, n], F32, name=f"fold_{g}", tag=f"fold_{g}")
            nc.gpsimd.dma_start(out=ft[:], in_=partials[C:2 * C, sl])
            ci = small.tile([C, n], F32, name=f"ccin_{g}", tag=f"ccin_{g}")
            nc.gpsimd.tensor_add(ci[:], partials[0:C, sl], ft[:])
            cid = dram.tile([C, n], F32, name=f"ccind_{g}", tag=f"ccind_{g}")
            cod = dram.tile([C, n], F32, name=f"ccoutd_{g}", tag=f"ccoutd_{g}")
            nc.gpsimd.dma_start(out=cid[:], in_=ci[:])
            groups = [[2 * i, 2 * i + 1] for i in range(n_cores // 2)]
            nc.gpsimd.collective_compute(
                "AllReduce", ALU.add, replica_groups=groups,
                ins=[cid[:].opt()], outs=[cod[:].opt()])
            cc_out_d[g] = (cod, n)

        def emit_cc_read(g):
            cod, n = cc_out_d[g]
            co = small.tile([C, n], F32, name=f"ccout_{g}", tag=f"ccout_{g}")
            nc.scalar.dma_start(out=co[:], in_=cod[:])
            cc_out_sb[g] = co

        for v in range(V):
            xv = xpool.tile([P, U, HL, W], BF16, tag="xv")
            xv_t.append(xv)
            nc.sync.dma_start(out=xv[:], in_=x_d[v])

            acc = ps_acc.tile([P, CHW], F32, tag="acc")   # sum over u, this v
            for u in range(U):
                for j0 in range(0, CHW, NMM):
                    mm16(acc[:, j0:j0 + NMM],
                         xv[:, u].rearrange("p hl w -> p (hl w)")[:, j0:j0 + NMM],
                         start=(u == 0), stop=(u == U - 1))
            # evacuate acc to SBUF bf16 (feeds hw accumulation + vw hl-fold)
            sumu = sumu_pool.tile([P, CHW], BF16, tag="sumu")
            nc.scalar.copy(out=sumu[:], in_=acc[:])
            # hw accumulation back through the PE
            for j0 in range(0, CHW, NMM):
                mm16(hw_ps[:, j0:j0 + NMM], sumu[:, j0:j0 + NMM],
                     start=(v == 0), stop=(v == V - 1))
            # vw partial: fold hl out of sumu via a GpSimd halving tree
            t1 = sumu_pool.tile([P, (HL // 2) * W], BF16, tag="vt1")
            nc.gpsimd.tensor_add(t1[:], sumu[:, 0:(HL // 2) * W],
                                 sumu[:, (HL // 2) * W:CHW])
            t2 = sumu_pool.tile([P, (HL // 4) * W], BF16, tag="vt2")
            nc.gpsimd.tensor_add(t2[:], t1[:, 0:(HL // 4) * W],
                                 t1[:, (HL // 4) * W:])
            t3 = sumu_pool.tile([P, (HL // 8) * W], BF16, tag="vt3")
            nc.gpsimd.tensor_add(t3[:], t2[:, 0:(HL // 8) * W],
                                 t2[:, (HL // 8) * W:])
            nc.gpsimd.tensor_add(partials[:, v * NB + U:(v + 1) * NB],
                                 t3[:, 0:W], t3[:, W:2 * W])
            # s_w (sum over w) for this v
            nc.vector.tensor_reduce(s_w[:, v], xv[:],
                                    axis=mybir.AxisListType.X, op=ALU.add)
            # uv partial for this v
            nc.vector.tensor_reduce(partials[:, v * NB:v * NB + U], s_w[:, v],
                                    axis=mybir.AxisListType.X, op=ALU.add)
            if v == VA - 1:
                emit_group_cc("A", 0, VA)
            elif v == V - 1:
                emit_group_cc("B", VA, V)

        # uh local sums -> means
        uh_raw = small.tile([P, U, HL], F32)
        swv = s_w[:].rearrange("p v u hl -> p u hl v")
        nc.vector.tensor_reduce(uh_raw[:], swv, axis=mybir.AxisListType.X,
                                op=ALU.add)
        uh_sc = small.tile([P, U * HL], BF16)
        nc.scalar.activation(out=uh_sc[:],
                             in_=uh_raw[:].rearrange("p u hl -> p (u hl)"),
                             func=AF.Copy, scale=1.0 / (V * W))
        # hw means
        hw_m = small.tile([P, CHW], BF16)
        nc.scalar.activation(out=hw_m[:], in_=hw_ps[:],
                             func=AF.Copy, scale=1.0 / NUV)
        phase1_ctx.close()   # release pool-phase PSUM banks
        ps1p = ctx.enter_context(tc.tile_pool(name="ps1p", bufs=2, space="PSUM"))
        ps2p = ctx.enter_context(tc.tile_pool(name="ps2p", bufs=2, space="PSUM"))
        ps3p = ctx.enter_context(tc.tile_pool(name="ps3p", bufs=2, space="PSUM"))
        psgp = ctx.enter_context(tc.tile_pool(name="psg", bufs=2, space="PSUM"))

        # gate buffers (same pixel orders as the conv inputs)
        hwg = small.tile([P, CHW], BF16)         # (hl, w) per (hs,c) partition
        uhg = small.tile([P, U * HL], BF16)      # (u, hl) per (hs,c) partition
        vwg = small.tile([P, V * W], BF16)       # (v, w), replicated over hs
        uvg = small.tile([P, NUV + 1], BF16)     # (v, u), replicated over hs
        uv_sc = small.tile([C, NUV + 1], BF16)   # (v,u) order (+1 pad col)
        vw_sc = small.tile([C, V * W], BF16)     # (v,w) order

        def run_conv_jobs(jobs):
            """Software-pipelined 1x1-conv chains (2 jobs in flight).

            Each job: (rhs_ap, nn, hs, f_nm, fb_nm, target, mirror). Chain:
            u1 = w1 @ rhs ; a1 = silu(u1 + b1) ; u2 = w2 @ a1 + b2 ;
            gate = f @ u2 + fb. For hs==0 the final ACT writes `target`
            directly (same partitions); hs==1 targets live on partitions
            64-127 so the gate goes through a bounce tile + DMA. `mirror`
            (optional) gets a DMA copy of `target`.
            """
            ps1s = [None] * len(jobs)
            for j in range(len(jobs) + 2):
                if j < len(jobs):
                    rhs, nn, hs, f_nm, fb_nm, target, mirror = jobs[j]
                    w_sl = slice(hs * C, (hs + 1) * C)
                    ps1 = ps1p.tile([C, nn], F32, tag="ps1")
                    mmw(ps1[:], wt["w1T"][w_sl, :], rhs)
                    ps1s[j] = ps1
                k = j - 2
                if k < 0 or k >= len(jobs):
                    continue
                rhs, nn, hs, f_nm, fb_nm, target, mirror = jobs[k]
                ps1 = ps1s[k]
                sig = convp.tile([C, nn], BF16, tag="sig")
                nc.scalar.activation(out=sig[:], in_=ps1[:], func=AF.Sigmoid,
                                     bias=bt["b1"])
                a1 = convp.tile([C, nn], BF16, tag="a1")
                nc.vector.scalar_tensor_tensor(
                    out=a1[:], in0=ps1[:], scalar=bt["b1"],
                    in1=sig[:], op0=ALU.add, op1=ALU.mult)
                ps2 = ps2p.tile([C, nn], F32, tag="ps2")
                mmw(ps2[:], wt["w2T"][0:C, :], a1[:])
                body = convp.tile([C, nn], BF16, tag="body")
                nc.vector.tensor_scalar(out=body[:], in0=ps2[:],
                                        scalar1=bt["b2"], scalar2=None,
                                        op0=ALU.add)
                ps3 = ps3p.tile([C, nn], F32, tag="ps3")
                mmw(ps3[:], wt[f_nm][0:C, :], body[:])
                if hs == 0:
                    nc.scalar.activation(out=target, in_=ps3[:],
                                         func=AF.Identity, bias=bt[fb_nm])
                    if mirror is not None:
                        nc.sync.dma_start(out=mirror, in_=target)
                else:
                    gate = convp.tile([C, nn], BF16, tag="gate")
                    nc.scalar.activation(out=gate[:], in_=ps3[:],
                                         func=AF.Identity, bias=bt[fb_nm])
                    nc.sync.dma_start(out=target, in_=gate[:])

        # local jobs: hw (per hs, per 512-chunk) and uh (per hs)
        jobs = []
        for hs in range(2):
            for j0 in range(0, CHW, NMM):
                jobs.append((hw_m[hs * C:(hs + 1) * C, j0:j0 + NMM], NMM, hs,
                             "f0T", "fb0",
                             hwg[hs * C:(hs + 1) * C, j0:j0 + NMM], None))
        for hs in range(2):
            jobs.append((uh_sc[hs * C:(hs + 1) * C], U * HL, hs,
                         "f2T", "fb2", uhg[hs * C:(hs + 1) * C, :], None))
        run_conv_jobs(jobs)

        # P_u = hwg + uhg[:, u, :] broadcast over w (local; before any cc dep)
        pbufs = []
        for u in range(U):
            pbuf = ppool.tile([P, CHW], BF16, tag="p")
            uh_b = _ap(uhg[:], [(1, HL), (0, W)], extra_off=u * HL)
            nc.vector.tensor_add(pbuf[:], hwg[:], uh_b)
            pbufs.append(pbuf)

        def scale_group(v0, v1, g):
            co = cc_out_sb[g]
            cnt = v1 - v0
            uv_src = _ap(co[:], [(NB, cnt), (1, U)])
            nc.vector.tensor_scalar(
                out=uv_sc[:, v0 * U:v1 * U], in0=uv_src,
                scalar1=1.0 / (H * W), scalar2=None, op0=ALU.mult)
            vw_src = _ap(co[:], [(NB, cnt), (1, W)], extra_off=U)
            nc.vector.tensor_scalar(
                out=vw_sc[:, v0 * W:v1 * W], in0=vw_src,
                scalar1=1.0 / (U * H), scalar2=None, op0=ALU.mult)
            if v1 == V and NUV % 2:   # pad col so uv widths stay even
                nc.vector.tensor_scalar(
                    out=uv_sc[:, NUV:NUV + 1], in0=co[:, 0:1],
                    scalar1=1.0, scalar2=None, op0=ALU.mult)

        def group_jobs(v0, v1):
            jb = []
            nvw = (v1 - v0) * W
            jb.append((vw_sc[:, v0 * W:v1 * W], nvw, 0, "f3T", "fb3",
                       vwg[0:C, v0 * W:v0 * W + nvw],
                       vwg[C:2 * C, v0 * W:v0 * W + nvw]))
            nuv = (v1 - v0) * U
            pad = nuv % 2
            jb.append((uv_sc[:, v0 * U:v1 * U + pad], nuv + pad, 0,
                       "f1T", "fb1",
                       uvg[0:C, v0 * U:v0 * U + nuv + pad],
                       uvg[C:2 * C, v0 * U:v0 * U + nuv + pad]))
            return jb

        def emit_q(v0, v1):
            cnt = v1 - v0
            vw_b = _ap(vwg[:], [(W, cnt), (0, U), (1, W)], extra_off=v0 * W)
            uv_b = _ap(uvg[:], [(U, cnt), (1, U), (0, W)], extra_off=v0 * U)
            nc.vector.tensor_add(qbuf[:, v0:v1], vw_b, uv_b)

        def emit_phase3(v0, v1):
            for v in range(v0, v1):
                obuf = opool.tile([P, U, HL, W], BF16, tag="obuf")
                for u in range(U):
                    xin = xv_t[v][:, u].rearrange("p hl w -> p (hl w)")
                    oin = obuf[:, u].rearrange("p hl w -> p (hl w)")
                    q_off = (v * U + u) * W
                    route = ROUTE_CYCLE[(v * U + u) % len(ROUTE_CYCLE)]
                    if route == "pe":
                        g = gpool.tile([P, CHW], BF16, tag="g_pe", bufs=3)
                        for hi in range(CHW // NMM):
                            ps = psgp.tile([P, NMM], F32, tag="gps")
                            mm16(ps[:], pbufs[u][:, hi * NMM:(hi + 1) * NMM],
                                 start=True, stop=False)
                            nc.tensor.matmul(
                                ps[:], id16,
                                _ap(qbuf[:], [(0, NMM // W), (1, W)],
                                    extra_off=q_off),
                                start=False, stop=True)
                            nc.scalar.copy(
                                out=g[:, hi * NMM:(hi + 1) * NMM], in_=ps[:])
                    elif route == "gp":
                        q_b = _ap(qbuf[:], [(0, HL), (1, W)], extra_off=q_off)
                        g = gpool.tile([P, CHW], BF16, tag="g_gp", bufs=3)
                        nc.gpsimd.tensor_add(g[:], pbufs[u][:], q_b)
                    else:
                        q_b = _ap(qbuf[:], [(0, HL), (1, W)], extra_off=q_off)
                        g = gpool.tile([P, CHW], BF16, tag="g_dve", bufs=2)
                        nc.vector.tensor_add(g[:], pbufs[u][:], q_b)
                    nc.vector.tensor_mul(oin, xin, g[:])
                nc.sync.dma_start(out=out_d[v], in_=obuf[:])

        qbuf = small.tile([P, V, U, W], BF16)

        # group A: scales + convs + Q, then its gating while B is in flight
        emit_cc_read("A")
        scale_group(0, VA, "A")
        run_conv_jobs(group_jobs(0, VA))
        emit_q(0, VA)
        emit_cc_read("B")
        emit_phase3(0, VA)
        # group B: scales + convs + Q + gating
        scale_group(VA, V, "B")
        run_conv_jobs(group_jobs(VA, V))
        emit_q(VA, V)
        emit_phase3(VA, V)

    nc.compile()
    return nc


# ---------------------------------------------------------------------------
# Host entry point (full problem size, 8 cores)

B, C, U, V, H, W = 4, 64, 5, 5, 64, 64
H2 = H // 2
HL = H2 // 2

_prog_cache = {}


def _get_prog():
    if "nc" not in _prog_cache:
        _prog_cache["nc"] = build_program(C=C, U=U, V=V, H2=H2, W=W, n_cores=8)
    return _prog_cache["nc"]


def make_const_pack(inputs):
    import ml_dtypes
    P = 2 * C
    ws = [np.asarray(inputs["w1"], np.float32).T,
          np.asarray(inputs["w2"], np.float32).T]
    bs = [np.asarray(inputs["b1"], np.float32),
          np.asarray(inputs["b2"], np.float32)]
    for i in range(4):
        ws.append(np.asarray(inputs[f"fw{i}"], np.float32).T)
        bs.append(np.asarray(inputs[f"fb{i}"], np.float32))
    # column layout: [ident(P) | 6 weights (C each, hs-replicated)]
    ncon = P + 6 * C
    cpack = np.zeros((P, ncon), dtype=np.float32)
    cpack[:, 0:P] = np.eye(P, dtype=np.float32)
    for i, w in enumerate(ws):
        cpack[:, P + i * C:P + (i + 1) * C] = np.vstack([w, w])
    cbias = np.zeros((C, 8), dtype=np.float32)
    for i, b in enumerate(bs):
        cbias[:, i] = b
    return cpack.astype(ml_dtypes.bfloat16), cbias


def make_in_maps(inputs):
    import ml_dtypes
    x = np.asarray(inputs["x"], dtype=np.float32)
    cpack, cbias = make_const_pack(inputs)
    base = {"cpack": cpack, "cbias": cbias}

    in_maps = []
    for core in range(8):
        b, hh = core // 2, core % 2
        s6 = x[b, :, :, :, hh * H2:(hh + 1) * H2, :]
        arr = np.ascontiguousarray(
            s6.reshape(C, U, V, 2, HL, W).transpose(2, 3, 0, 1, 4, 5))
        arr = arr.reshape(V, 2 * C, U, HL, W).astype(ml_dtypes.bfloat16)
        in_maps.append({"x": arr, **base})
    return in_maps


def assemble_out(results):
    out = np.empty((B, C, U, V, H, W), dtype=np.float32)
    for core in range(8):
        b, hh = core // 2, core % 2
        r = np.asarray(results[core]["out"]).astype(np.float32)
        r = r.reshape(V, 2, C, U, HL, W)
        out[b, :, :, :, hh * H2:(hh + 1) * H2, :] = (
            r.transpose(2, 3, 0, 1, 4, 5).reshape(C, U, V, H2, W))
    return out


def kernel(**inputs):
    from concourse.bass_utils import run_bass_kernel_spmd

    in_maps = make_in_maps(inputs)
    nc = _get_prog()
    res = run_bass_kernel_spmd(nc, in_maps, core_ids=list(range(8)))
    return assemble_out(res.results)


# revision 28
# speedup vs baseline: 1.0986x; 1.0986x over previous
"""Trainium2 Bass kernel for the MCA (multi-axis pooled gating) module.

Computation (per sample b):
    hw_m = mean_{u,v} x   uv_m = mean_{h,w} x   uh_m = mean_{v,w} x   vw_m = mean_{u,h} x
    body = conv2(silu(conv1(uvhw)))   (1x1 convs on the packed (H+V, W+U) pooled map)
    gates: hw_g = f0(body_hw), uv_g = f1(body_uv), uh_g = f2(body_uh), vw_g = f3(body_vw)
    out = x * (hw_g + uv_g + uh_g + vw_g)      (each gate broadcast to the 6D shape)

Distribution: 8 cores = 4 samples x 2 h-halves. Each core owns
x[b, :, :, :, hh*32:(hh+1)*32, :], held in SBUF as bf16 (the host converts; the
2e-2 tolerance dwarfs bf16 rounding), so HBM traffic is halved in both
directions versus f32. The only cross-core data are the h-reduced pools
(uv_m, vw_m partials), exchanged as two small pair AllReduces (split by v so
the second half overlaps the first group's gating work).

On-core layout: SBUF partition p = hs*64 + c, where the core's 32 h-rows split
as h2 = hs*16 + hl. Pools that fully reduce h fold the hs partition halves with
a small DMA+add before the collectives.

Engine budget: PE does the (u,v)-pool accumulation (bf16 identity matmuls),
the vw hl-fold, one v's w-reduction, the channel-mixing convs and the B-group
gate-broadcast adds; DVE does the remaining w-reductions, the final multiplies
and a share of the gate adds; GpSimd takes the other gate adds plus collective
staging; ACT does PSUM evacuation, scaling, SiLU and gate biases.
"""

import sys
if '/opt/trn_rl_repo' not in sys.path:
    sys.path.insert(0, '/opt/trn_rl_repo')

from contextlib import ExitStack

import numpy as np
import concourse.bass as bass
import concourse.bacc as bacc
import concourse.tile as tile
from concourse import mybir

F32 = mybir.dt.float32
F32R = mybir.dt.float32r
BF16 = mybir.dt.bfloat16
AF = mybir.ActivationFunctionType
ALU = mybir.AluOpType

# ---- tunable routing -------------------------------------------------------
SW_PE_VS = ()              # v indices whose w-reduction runs on PE (rest DVE)
# per-chunk G-add route, cycled over the 25 (v,u) chunks. GpSimd elementwise
# work contends with DVE 2-port reads (shared SBUF port), so keep gp light.
ROUTE_CYCLE = ("pe", "gp", "dve", "pe", "gp", "pe",
               "dve", "pe", "gp", "pe", "dve", "pe")


def _ap(t_ap, dims, extra_off=0):
    """Manual free-dim view of an AP: dims = [(step_elems, count), ...]."""
    return bass.AP(
        tensor=t_ap.tensor,
        offset=t_ap.offset + extra_off,
        ap=[list(t_ap.ap[0])] + [[s, c] for (s, c) in dims],
    )


def build_program(C=64, U=5, V=5, H2=32, W=64, n_cores=8):
    """One SPMD program; per-core inputs select the (b, h-half) shard."""
    assert C == 64 and H2 % 2 == 0
    HL = H2 // 2              # h rows per hs partition group
    P = 2 * C                 # 128 partitions = (hs, c)
    CHW = HL * W              # free size of one (u,v) chunk per partition
    NMM = min(512, CHW)       # matmul moving-operand max (PSUM bank)
    NUV = U * V
    NB = U + W                # per-v partials block: [uv_u | vw_w]
    VA = max(1, (3 * V) // 5)  # v-count in the first collective group
    H = 2 * H2
    FREE = U * V * HL * W // V  # per-v free size = U*HL*W

    nc = bacc.Bacc('TRN2', target_bir_lowering=False, debug=False,
                   enable_asserts=False, num_devices=n_cores)

    x_d = nc.dram_tensor("x", [V, P, U, HL, W], BF16, kind="ExternalInput").ap()
    out_d = nc.dram_tensor("out", [V, P, U, HL, W], BF16,
                           kind="ExternalOutput").ap()
    NCON = P + 6 * C
    cpack_d = nc.dram_tensor("cpack", [P, NCON], BF16, kind="ExternalInput").ap()
    cbias_d = nc.dram_tensor("cbias", [C, 8], F32, kind="ExternalInput").ap()

    with tile.TileContext(nc) as tc, ExitStack() as ctx:
        consts = ctx.enter_context(tc.tile_pool(name="consts", bufs=1))
        xpool = ctx.enter_context(tc.tile_pool(name="x", bufs=V))
        sumu_pool = ctx.enter_context(tc.tile_pool(name="sumu", bufs=2))
        small = ctx.enter_context(tc.tile_pool(name="small", bufs=1))
        convp = ctx.enter_context(tc.tile_pool(name="convp", bufs=2))
        ppool = ctx.enter_context(tc.tile_pool(name="pp", bufs=U))
        gpool = ctx.enter_context(tc.tile_pool(name="gpool", bufs=3))
        opool = ctx.enter_context(tc.tile_pool(name="opool", bufs=2))
        phase1_ctx = ExitStack()
        ps_acc = phase1_ctx.enter_context(
            tc.tile_pool(name="ps_acc", bufs=3, space="PSUM"))
        ps_hw = phase1_ctx.enter_context(
            tc.tile_pool(name="ps_hw", bufs=1, space="PSUM"))
        dram = ctx.enter_context(tc.tile_pool(name="dram", bufs=1, space="DRAM"))

        cpack = consts.tile([P, NCON], BF16)
        nc.gpsimd.dma_start(out=cpack[:], in_=cpack_d[:, :])
        cbias = consts.tile([C, 8], F32)
        nc.gpsimd.dma_start(out=cbias[:], in_=cbias_d[:, :])
        id16 = cpack[:, 0:P]
        # weights replicated on both hs partition halves so conv matmuls can
        # pick an lhsT whose base partition matches the rhs half
        wnames = ("w1T", "w2T", "f0T", "f1T", "f2T", "f3T")
        wt = {nm: cpack[:, P + i * C:P + (i + 1) * C]
              for i, nm in enumerate(wnames)}
        bnames = ("b1", "b2", "fb0", "fb1", "fb2", "fb3")
        bt = {nm: cbias[0:C, i:i + 1] for i, nm in enumerate(bnames)}

        def mm16(out_ps, rhs, start, stop):
            nc.tensor.matmul(out_ps, id16, rhs, start=start, stop=stop)

        def mmw(out_ps, lhsT, rhs, start=True, stop=True):
            nc.tensor.matmul(out_ps, lhsT, rhs, start=start, stop=stop)

        # ---------------- Phase 1: load x + pools -------------------------
        partials = small.tile([P, V * NB], F32)   # per-v blocks [uv_u | vw_w]
        s_w = small.tile([P, V, U, HL], F32)      # x summed over w
        hw_ps = ps_hw.tile([P, CHW], F32)         # x summed over (u, v)
        xv_t = []

        cc_out_sb = {}
        cc_out_d = {}

        def emit_group_cc(g, v0, v1):
            """Fold hs halves of partials[v0:v1]; trigger the pair AllReduce."""
            sl = slice(v0 * NB, v1 * NB)
            n = (v1 - v0) * NB
            ft = small.tile([C, n], F32, name=f"fold_{g}", tag=f"fold_{g}")
            nc.gpsimd.dma_start(out=ft[:], in_=partials[C:2 * C, sl])
            ci = small.tile([C, n], F32, name=f"ccin_{g}", tag=f"ccin_{g}")
            nc.gpsimd.tensor_add(ci[:], partials[0:C, sl], ft[:])
            cid = dram.tile([C, n], F32, name=f"ccind_{g}", tag=f"ccind_{g}")
            cod = dram.tile([C, n], F32, name=f"ccoutd_{g}", tag=f"ccoutd_{g}")
            nc.gpsimd.dma_start(out=cid[:], in_=ci[:])
            groups = [[2 * i, 2 * i + 1] for i in range(n_cores // 2)]
            nc.gpsimd.collective_compute(
                "AllReduce", ALU.add, replica_groups=groups,
                ins=[cid[:].opt()], outs=[cod[:].opt()])
            cc_out_d[g] = (cod, n)

        def emit_cc_read(g):
            cod, n = cc_out_d[g]
            co = small.tile([C, n], F32, name=f"ccout_{g}", tag=f"ccout_{g}")
            nc.scalar.dma_start(out=co[:], in_=cod[:])
            cc_out_sb[g] = co

        for v in range(V):
            xv = xpool.tile([P, U, HL, W], BF16, tag="xv")
            xv_t.append(xv)
            nc.sync.dma_start(out=xv[:], in_=x_d[v])

            acc = ps_acc.tile([P, CHW], F32, tag="acc")   # sum over u, this v
            for u in range(U):
                for j0 in range(0, CHW, NMM):
                    mm16(acc[:, j0:j0 + NMM],
                         xv[:, u].rearrange("p hl w -> p (hl w)")[:, j0:j0 + NMM],
                         start=(u == 0), stop=(u == U - 1))
            # evacuate acc to SBUF bf16 (feeds hw accumulation + vw hl-fold)
            sumu = sumu_pool.tile([P, CHW], BF16, tag="sumu")
            nc.scalar.copy(out=sumu[:], in_=acc[:])
            # hw accumulation back through the PE
            for j0 in range(0, CHW, NMM):
                mm16(hw_ps[:, j0:j0 + NMM], sumu[:, j0:j0 + NMM],
                     start=(v == 0), stop=(v == V - 1))
            # vw partial: fold hl out of sumu via a GpSimd halving tree
            t1 = sumu_pool.tile([P, (HL // 2) * W], BF16, tag="vt1")
            nc.gpsimd.tensor_add(t1[:], sumu[:, 0:(HL // 2) * W],
                                 sumu[:, (HL // 2) * W:CHW])
            t2 = sumu_pool.tile([P, (HL // 4) * W], BF16, tag="vt2")
            nc.gpsimd.tensor_add(t2[:], t1[:, 0:(HL // 4) * W],
                                 t1[:, (HL // 4) * W:])
            t3 = sumu_pool.tile([P, (HL // 8) * W], BF16, tag="vt3")
            nc.gpsimd.tensor_add(t3[:], t2[:, 0:(HL // 8) * W],
                                 t2[:, (HL // 8) * W:])
            nc.gpsimd.tensor_add(partials[:, v * NB + U:(v + 1) * NB],
                                 t3[:, 0:W], t3[:, W:2 * W])
            # s_w (sum over w) for this v: two dense bf16 pair-folds (DVE 2x
            # mode) then a 1x reduce over the remaining 16 columns
            f1 = sumu_pool.tile([P, U, HL, W // 2], BF16, tag="swf1")
            nc.vector.tensor_add(
                f1[:], _ap(xv[:], [(HL * W, U), (W, HL), (1, W // 2)]),
                _ap(xv[:], [(HL * W, U), (W, HL), (1, W // 2)],
                    extra_off=W // 2))
            f2 = sumu_pool.tile([P, U, HL, W // 4], BF16, tag="swf2")
            nc.vector.tensor_add(
                f2[:], _ap(f1[:], [(HL * W // 2, U), (W // 2, HL), (1, W // 4)]),
                _ap(f1[:], [(HL * W // 2, U), (W // 2, HL), (1, W // 4)],
                    extra_off=W // 4))
            nc.vector.tensor_reduce(s_w[:, v], f2[:],
                                    axis=mybir.AxisListType.X, op=ALU.add)
            # uv partial for this v
            nc.vector.tensor_reduce(partials[:, v * NB:v * NB + U], s_w[:, v],
                                    axis=mybir.AxisListType.X, op=ALU.add)
            if v == V - 1:
                emit_group_cc("A", 0, V)

        # uh local sums -> means
        uh_raw = small.tile([P, U, HL], F32)
        swv = s_w[:].rearrange("p v u hl -> p u hl v")
        nc.vector.tensor_reduce(uh_raw[:], swv, axis=mybir.AxisListType.X,
                                op=ALU.add)
        uh_sc = small.tile([P, U * HL], BF16)
        nc.scalar.activation(out=uh_sc[:],
                             in_=uh_raw[:].rearrange("p u hl -> p (u hl)"),
                             func=AF.Copy, scale=1.0 / (V * W))
        # hw means
        hw_m = small.tile([P, CHW], BF16)
        nc.scalar.activation(out=hw_m[:], in_=hw_ps[:],
                             func=AF.Copy, scale=1.0 / NUV)
        phase1_ctx.close()   # release pool-phase PSUM banks
        ps1p = ctx.enter_context(tc.tile_pool(name="ps1p", bufs=2, space="PSUM"))
        ps2p = ctx.enter_context(tc.tile_pool(name="ps2p", bufs=2, space="PSUM"))
        ps3p = ctx.enter_context(tc.tile_pool(name="ps3p", bufs=2, space="PSUM"))
        psgp = ctx.enter_context(tc.tile_pool(name="psg", bufs=2, space="PSUM"))

        # gate buffers (same pixel orders as the conv inputs)
        hwg = small.tile([P, CHW], BF16)         # (hl, w) per (hs,c) partition
        uhg = small.tile([P, U * HL], BF16)      # (u, hl) per (hs,c) partition
        vwg = small.tile([P, V * W], BF16)       # (v, w), replicated over hs
        uvg = small.tile([P, NUV + 1], BF16)     # (v, u), replicated over hs
        uv_sc = small.tile([C, NUV + 1], BF16)   # (v,u) order (+1 pad col)
        vw_sc = small.tile([C, V * W], BF16)     # (v,w) order

        def run_conv_jobs(jobs):
            """Software-pipelined 1x1-conv chains (2 jobs in flight).

            Each job: (rhs_ap, nn, hs, f_nm, fb_nm, target, mirror). Chain:
            u1 = w1 @ rhs ; a1 = silu(u1 + b1) ; u2 = w2 @ a1 + b2 ;
            gate = f @ u2 + fb. For hs==0 the final ACT writes `target`
            directly (same partitions); hs==1 targets live on partitions
            64-127 so the gate goes through a bounce tile + DMA. `mirror`
            (optional) gets a DMA copy of `target`.
            """
            ps1s = [None] * len(jobs)
            for j in range(len(jobs) + 2):
                if j < len(jobs):
                    rhs, nn, hs, f_nm, fb_nm, target, mirror = jobs[j]
                    w_sl = slice(hs * C, (hs + 1) * C)
                    ps1 = ps1p.tile([C, nn], F32, tag="ps1")
                    mmw(ps1[:], wt["w1T"][w_sl, :], rhs)
                    ps1s[j] = ps1
                k = j - 2
                if k < 0 or k >= len(jobs):
                    continue
                rhs, nn, hs, f_nm, fb_nm, target, mirror = jobs[k]
                ps1 = ps1s[k]
                sig = convp.tile([C, nn], BF16, tag="sig")
                nc.scalar.activation(out=sig[:], in_=ps1[:], func=AF.Sigmoid,
                                     bias=bt["b1"])
                a1 = convp.tile([C, nn], BF16, tag="a1")
                nc.vector.scalar_tensor_tensor(
                    out=a1[:], in0=ps1[:], scalar=bt["b1"],
                    in1=sig[:], op0=ALU.add, op1=ALU.mult)
                ps2 = ps2p.tile([C, nn], F32, tag="ps2")
                mmw(ps2[:], wt["w2T"][0:C, :], a1[:])
                body = convp.tile([C, nn], BF16, tag="body")
                nc.vector.tensor_scalar(out=body[:], in0=ps2[:],
                                        scalar1=bt["b2"], scalar2=None,
                                        op0=ALU.add)
                ps3 = ps3p.tile([C, nn], F32, tag="ps3")
                mmw(ps3[:], wt[f_nm][0:C, :], body[:])
                if hs == 0:
                    nc.scalar.activation(out=target, in_=ps3[:],
                                         func=AF.Identity, bias=bt[fb_nm])
                    if mirror is not None:
                        nc.sync.dma_start(out=mirror, in_=target)
                else:
                    gate = convp.tile([C, nn], BF16, tag="gate")
                    nc.scalar.activation(out=gate[:], in_=ps3[:],
                                         func=AF.Identity, bias=bt[fb_nm])
                    nc.sync.dma_start(out=target, in_=gate[:])

        # local jobs: hw (per hs, per 512-chunk) and uh (per hs)
        jobs = []
        for hs in range(2):
            for j0 in range(0, CHW, NMM):
                jobs.append((hw_m[hs * C:(hs + 1) * C, j0:j0 + NMM], NMM, hs,
                             "f0T", "fb0",
                             hwg[hs * C:(hs + 1) * C, j0:j0 + NMM], None))
        for hs in range(2):
            jobs.append((uh_sc[hs * C:(hs + 1) * C], U * HL, hs,
                         "f2T", "fb2", uhg[hs * C:(hs + 1) * C, :], None))
        run_conv_jobs(jobs)

        # P_u = hwg + uhg[:, u, :] broadcast over w (local; before any cc dep)
        pbufs = []
        for u in range(U):
            pbuf = ppool.tile([P, CHW], BF16, tag="p")
            uh_b = _ap(uhg[:], [(1, HL), (0, W)], extra_off=u * HL)
            nc.vector.tensor_add(pbuf[:], hwg[:], uh_b)
            pbufs.append(pbuf)

        def scale_group(v0, v1, g):
            co = cc_out_sb[g]
            cnt = v1 - v0
            uv_src = _ap(co[:], [(NB, cnt), (1, U)])
            nc.vector.tensor_scalar(
                out=uv_sc[:, v0 * U:v1 * U], in0=uv_src,
                scalar1=1.0 / (H * W), scalar2=None, op0=ALU.mult)
            vw_src = _ap(co[:], [(NB, cnt), (1, W)], extra_off=U)
            nc.vector.tensor_scalar(
                out=vw_sc[:, v0 * W:v1 * W], in0=vw_src,
                scalar1=1.0 / (U * H), scalar2=None, op0=ALU.mult)
            if ((v1 - v0) * U) % 2:   # pad col so uv widths stay even
                nc.vector.tensor_scalar(
                    out=uv_sc[:, v1 * U:v1 * U + 1], in0=co[:, 0:1],
                    scalar1=1.0, scalar2=None, op0=ALU.mult)

        def group_jobs(v0, v1):
            jb = []
            nvw = (v1 - v0) * W
            jb.append((vw_sc[:, v0 * W:v1 * W], nvw, 0, "f3T", "fb3",
                       vwg[0:C, v0 * W:v0 * W + nvw],
                       vwg[C:2 * C, v0 * W:v0 * W + nvw]))
            nuv = (v1 - v0) * U
            pad = nuv % 2
            jb.append((uv_sc[:, v0 * U:v1 * U + pad], nuv + pad, 0,
                       "f1T", "fb1",
                       uvg[0:C, v0 * U:v0 * U + nuv + pad],
                       uvg[C:2 * C, v0 * U:v0 * U + nuv + pad]))
            return jb

        def emit_q(v0, v1):
            cnt = v1 - v0
            vw_b = _ap(vwg[:], [(W, cnt), (0, U), (1, W)], extra_off=v0 * W)
            uv_b = _ap(uvg[:], [(U, cnt), (1, U), (0, W)], extra_off=v0 * U)
            nc.vector.tensor_add(qbuf[:, v0:v1], vw_b, uv_b)

        def emit_phase3(v0, v1):
            for v in range(v0, v1):
                obuf = opool.tile([P, U, HL, W], BF16, tag="obuf")
                for u in range(U):
                    xin = xv_t[v][:, u].rearrange("p hl w -> p (hl w)")
                    oin = obuf[:, u].rearrange("p hl w -> p (hl w)")
                    q_off = (v * U + u) * W
                    route = ROUTE_CYCLE[(v * U + u) % len(ROUTE_CYCLE)]
                    if route == "pe":
                        g = gpool.tile([P, CHW], BF16, tag="g_pe", bufs=3)
                        for hi in range(CHW // NMM):
                            ps = psgp.tile([P, NMM], F32, tag="gps")
                            mm16(ps[:], pbufs[u][:, hi * NMM:(hi + 1) * NMM],
                                 start=True, stop=False)
                            nc.tensor.matmul(
                                ps[:], id16,
                                _ap(qbuf[:], [(0, NMM // W), (1, W)],
                                    extra_off=q_off),
                                start=False, stop=True)
                            nc.scalar.copy(
                                out=g[:, hi * NMM:(hi + 1) * NMM], in_=ps[:])
                    elif route == "gp":
                        q_b = _ap(qbuf[:], [(0, HL), (1, W)], extra_off=q_off)
                        g = gpool.tile([P, CHW], BF16, tag="g_gp", bufs=3)
                        nc.gpsimd.tensor_add(g[:], pbufs[u][:], q_b)
                    else:
                        q_b = _ap(qbuf[:], [(0, HL), (1, W)], extra_off=q_off)
                        g = gpool.tile([P, CHW], BF16, tag="g_dve", bufs=2)
                        nc.vector.tensor_add(g[:], pbufs[u][:], q_b)
                    nc.vector.tensor_mul(oin, xin, g[:])
                nc.sync.dma_start(out=out_d[v], in_=obuf[:])

        qbuf = small.tile([P, V, U, W], BF16)

        emit_cc_read("A")
        scale_group(0, V, "A")
        run_conv_jobs(group_jobs(0, V))
        emit_q(0, V)
        emit_phase3(0, V)

    nc.compile()
    return nc


# ---------------------------------------------------------------------------
# Host entry point (full problem size, 8 cores)

B, C, U, V, H, W = 4, 64, 5, 5, 64, 64
H2 = H // 2
HL = H2 // 2

_prog_cache = {}


def _get_prog():
    if "nc" not in _prog_cache:
        _prog_cache["nc"] = build_program(C=C, U=U, V=V, H2=H2, W=W, n_cores=8)
    return _prog_cache["nc"]


def make_const_pack(inputs):
    import ml_dtypes
    P = 2 * C
    ws = [np.asarray(inputs["w1"], np.float32).T,
          np.asarray(inputs["w2"], np.float32).T]
    bs = [np.asarray(inputs["b1"], np.float32),
          np.asarray(inputs["b2"], np.float32)]
    for i in range(4):
        ws.append(np.asarray(inputs[f"fw{i}"], np.float32).T)
        bs.append(np.asarray(inputs[f"fb{i}"], np.float32))
    # column layout: [ident(P) | 6 weights (C each, hs-replicated)]
    ncon = P + 6 * C
    cpack = np.zeros((P, ncon), dtype=np.float32)
    cpack[:, 0:P] = np.eye(P, dtype=np.float32)
    for i, w in enumerate(ws):
        cpack[:, P + i * C:P + (i + 1) * C] = np.vstack([w, w])
    cbias = np.zeros((C, 8), dtype=np.float32)
    for i, b in enumerate(bs):
        cbias[:, i] = b
    return cpack.astype(ml_dtypes.bfloat16), cbias


def make_in_maps(inputs):
    import ml_dtypes
    x = np.asarray(inputs["x"], dtype=np.float32)
    cpack, cbias = make_const_pack(inputs)
    base = {"cpack": cpack, "cbias": cbias}

    in_maps = []
    for core in range(8):
        b, hh = core // 2, core % 2
        s6 = x[b, :, :, :, hh * H2:(hh + 1) * H2, :]
        arr = np.ascontiguousarray(
            s6.reshape(C, U, V, 2, HL, W).transpose(2, 3, 0, 1, 4, 5))
        arr = arr.reshape(V, 2 * C, U, HL, W).astype(ml_dtypes.bfloat16)
        in_maps.append({"x": arr, **base})
    return in_maps


def assemble_out(results):
    out = np.empty((B, C, U, V, H, W), dtype=np.float32)
    for core in range(8):
        b, hh = core // 2, core % 2
        r = np.asarray(results[core]["out"]).astype(np.float32)
        r = r.reshape(V, 2, C, U, HL, W)
        out[b, :, :, :, hh * H2:(hh + 1) * H2, :] = (
            r.transpose(2, 3, 0, 1, 4, 5).reshape(C, U, V, H2, W))
    return out


def kernel(**inputs):
    from concourse.bass_utils import run_bass_kernel_spmd

    in_maps = make_in_maps(inputs)
    nc = _get_prog()
    res = run_bass_kernel_spmd(nc, in_maps, core_ids=list(range(8)))
    return assemble_out(res.results)
